# revision 15
# baseline (speedup 1.0000x reference)
"""Trainium2 Bass kernel for EuclideanSimilarity (retrieval_knn).

Reference computation per batch b (B=8, L=4096, D=128):
    projected = x @ W.T + b                      [L, D]
    q = avgpool2(x) @ W.T + b                    [L/2, D]   (== avgpool2(projected))
    power = ||q_i||^2 + ||k_j||^2 - 2 q_i.k_j    [L/2, L]
    sim = exp(-sqrt(max(power, 0)))
    k = sim @ projected                          [L/2, D]
    returns (q, k, v=k)

Sharding: data-parallel over batch, one batch element per NeuronCore (8 cores).
All device tensors keep the feature dim D=128 on SBUF partitions where the
matmuls contract over it; host pre-transposes x and post-transposes q/k
(host-side layout prep is free wrt HW exec time).

Per-core pipeline (all fp32):
  projT_m2[e,l] = (-2W)^T x + (-2b)      8 matmuls, stationary -2W^T
  projnat[l,e]  = x_tile^T W^T + b       32 matmuls, xT tiles stationary
  ksq[j]        = sum_e projnat^2        DVE square+reduce
  qT[e,i]       = (W/2)^T (x_even+x_odd) + b
  qsq_bcast     = ones^T @ qT^2          reduce+partition-broadcast in one matmul
  per 1024-query chunk:
    psum  = -2 q.k                       (GEMM2, projT_m2 tiles stationary)
    power = psum + ksq[j] + qsq[i]       one fused DVE op (affine_then_add)
    sim   = Exp(-Sqrt(power))            two ACT passes over 32K-column strip
    kT   += projnat_jt @ sim_strip       (GEMM3, accumulated over 32 j-tiles)
"""

import os
import sys

for _p in ("/opt/trn_rl_repo", "/root/.axon_site/_ro/trn_rl_repo"):
    if os.path.isdir(_p) and _p not in sys.path:
        sys.path.insert(0, _p)

import numpy as np

import concourse.bass as bass
import concourse.mybir as mybir
from concourse import bacc
from concourse.bass_utils import run_bass_kernel_spmd
from concourse.tile import TileContext

B, L, D = 8, 4096, 128
LQ = L // 2          # 2048 pooled queries
P = 128              # partitions
NI = 1024            # i-chunk (queries per chunk)
NCHUNK = LQ // NI    # 2
NJT = L // P         # 32 j-tiles
F32 = mybir.dt.float32

AF = mybir.ActivationFunctionType
ALU = mybir.AluOpType


def build_nc(repeat=1):
    nc = bacc.Bacc("TRN2", target_bir_lowering=False)

    xT = nc.declare_dram_parameter("xT", [P, L], F32, isOutput=False)
    WT = nc.declare_dram_parameter("WT", [P, D], F32, isOutput=False)       # W.T
    Wm2T = nc.declare_dram_parameter("Wm2T", [P, D], F32, isOutput=False)   # (-2W).T
    WhT = nc.declare_dram_parameter("WhT", [P, D], F32, isOutput=False)     # (0.5W).T
    bcols = nc.declare_dram_parameter("bcols", [P, 2], F32, isOutput=False)  # [b, -2b]
    b_bcast_in = nc.declare_dram_parameter("b_bcast", [P, D], F32, isOutput=False)
    ones_in = nc.declare_dram_parameter("ones_mat", [P, P], F32, isOutput=False)

    qT_out = nc.declare_dram_parameter("qT", [P, LQ], F32, isOutput=True)
    kT_out = nc.declare_dram_parameter("kT", [P, LQ], F32, isOutput=True)

    with TileContext(nc) as tc:
      for _rep in range(repeat):
        with (
            tc.tile_pool(name="consts", bufs=1) as consts,
            tc.tile_pool(name="big", bufs=1) as big,
            tc.tile_pool(name="work", bufs=2) as work,
            tc.tile_pool(name="ps1", bufs=2, space="PSUM") as ps1,
        ):
            # ---- constants ----
            WT_sb = consts.tile([P, D], F32)
            Wm2T_sb = consts.tile([P, D], F32)
            WhT_sb = consts.tile([P, D], F32)
            bcols_sb = consts.tile([P, 2], F32)
            b_bcast = consts.tile([P, D], F32)
            ones_sb = consts.tile([P, P], F32)
            nc.sync.dma_start(out=WT_sb[:], in_=WT[:])
            nc.sync.dma_start(out=Wm2T_sb[:], in_=Wm2T[:])
            nc.sync.dma_start(out=WhT_sb[:], in_=WhT[:])
            nc.sync.dma_start(out=bcols_sb[:], in_=bcols[:])
            nc.sync.dma_start(out=b_bcast[:], in_=b_bcast_in[:])
            nc.sync.dma_start(out=ones_sb[:], in_=ones_in[:])
            b_col = bcols_sb[:, 0:1]
            bm2_col = bcols_sb[:, 1:2]

            projTm2 = big.tile([P, L], F32)
            projnat = big.tile([P, L], F32)  # 32 tiles of [128, 128] along free
            qT_sb = big.tile([P, LQ], F32)
            qsq_bcast = big.tile([P, LQ], F32)
            ksq = consts.tile([P, NJT], F32)

            # ---- phase 1 (xT-dependent); pool closed before strip opens ----
            with tc.tile_pool(name="phase1", bufs=1) as ph1:
                xT_sb = ph1.tile([P, L], F32)
                nc.sync.dma_start(out=xT_sb[:], in_=xT[:])
                xT_pairs = xT_sb.rearrange("p (i two) -> p i two", two=2)
                xpool = ph1.tile([P, LQ], F32)
                nc.vector.tensor_add(xpool[:], xT_pairs[:, :, 0], xT_pairs[:, :, 1])

                # projT_m2[e, l] = -2 * (W x + b)^T
                for c in range(L // 512):
                    ps = ps1.tile([P, 512], F32, tag="ps1")
                    nc.tensor.matmul(
                        ps, Wm2T_sb[:], xT_sb[:, c * 512:(c + 1) * 512],
                        start=True, stop=True,
                    )
                    nc.vector.tensor_scalar_add(
                        projTm2[:, c * 512:(c + 1) * 512], ps, bm2_col)

                # proj_nat tiles [l(128), e] and ksq
                for t in range(NJT):
                    ps = ps1.tile([P, D], F32, tag="ps1")
                    nc.tensor.matmul(
                        ps, xT_sb[:, t * P:(t + 1) * P], WT_sb[:],
                        start=True, stop=True,
                    )
                    seg = projnat[:, t * P:(t + 1) * P]
                    nc.vector.tensor_add(seg, ps, b_bcast[:])
                    sq = work.tile([P, D], F32, tag="sqs")
                    nc.vector.tensor_mul(sq[:], seg, seg)
                    nc.vector.tensor_reduce(
                        ksq[:, t:t + 1], sq[:], mybir.AxisListType.X, ALU.add)

                # qT[e, i] = 0.5*W @ xpool + b
                for c in range(LQ // 512):
                    ps = ps1.tile([P, 512], F32, tag="ps1")
                    nc.tensor.matmul(
                        ps, WhT_sb[:], xpool[:, c * 512:(c + 1) * 512],
                        start=True, stop=True,
                    )
                    nc.vector.tensor_scalar_add(
                        qT_sb[:, c * 512:(c + 1) * 512], ps, b_col)
                nc.sync.dma_start(out=qT_out[:], in_=qT_sb[:])

                # qsq_bcast[p, i] = ||q_i||^2 broadcast to all partitions:
                # all-ones stationary does reduce + broadcast in one matmul.
                sq_qT = ph1.tile([P, LQ], F32)
                nc.vector.tensor_mul(sq_qT[:], qT_sb[:], qT_sb[:])
                for c in range(LQ // 512):
                    ps = ps1.tile([P, 512], F32, tag="ps1")
                    nc.tensor.matmul(
                        ps, ones_sb[:], sq_qT[:, c * 512:(c + 1) * 512],
                        start=True, stop=True,
                    )
                    nc.scalar.copy(qsq_bcast[:, c * 512:(c + 1) * 512], ps)

            # ---- main loop over query chunks ----
            with (
                tc.tile_pool(name="stripp", bufs=1) as stripp,
                tc.tile_pool(name="psqk", bufs=2, space="PSUM") as psqk,
                tc.tile_pool(name="psk", bufs=1, space="PSUM") as psk,
            ):
                for c in range(NCHUNK):
                    strip = stripp.tile([P, NJT * NI], F32, tag="strip")
                    qs = qsq_bcast[:, c * NI:(c + 1) * NI]
                    for jt in range(NJT):
                        ps2 = psqk.tile([P, NI], F32, tag="qk")
                        for h in range(NI // 512):
                            nc.tensor.matmul(
                                ps2[:, h * 512:(h + 1) * 512],
                                projTm2[:, jt * P:(jt + 1) * P],
                                qT_sb[:, c * NI + h * 512:c * NI + (h + 1) * 512],
                                start=True, stop=True,
                            )
                        # power = (-2qk) + ksq[j] + qsq[i], one fused DVE op
                        nc.vector.affine_then_add(
                            strip[:, jt * NI:(jt + 1) * NI], ps2, qs,
                            scale=1.0, bias=ksq[:, jt:jt + 1],
                        )
                    nc.scalar.activation(strip[:], strip[:], AF.Sqrt)
                    nc.scalar.activation(strip[:], strip[:], AF.Exp, scale=-1.0)
                    ps3 = psk.tile([P, NI], F32, tag="kacc")
                    for jt in range(NJT):
                        for h in range(NI // 512):
                            nc.tensor.matmul(
                                ps3[:, h * 512:(h + 1) * 512],
                                projnat[:, jt * P:(jt + 1) * P],
                                strip[:, jt * NI + h * 512:jt * NI + (h + 1) * 512],
                                start=(jt == 0), stop=(jt == NJT - 1),
                            )
                    kT_tile = work.tile([P, NI], F32, tag="kout")
                    nc.vector.tensor_copy(kT_tile[:], ps3)
                    nc.sync.dma_start(
                        out=kT_out[:, c * NI:(c + 1) * NI], in_=kT_tile[:])

    nc.compile()
    return nc


_NC_CACHE = {}


def _get_nc():
    if "nc" not in _NC_CACHE:
        _NC_CACHE["nc"] = build_nc()
    return _NC_CACHE["nc"]


def kernel(x, W, b):
    x = np.asarray(x, dtype=np.float32)
    W = np.asarray(W, dtype=np.float32)
    b = np.asarray(b, dtype=np.float32)

    nc = _get_nc()

    WT = np.ascontiguousarray(W.T)
    Wm2T = np.ascontiguousarray((-2.0 * W).T)
    WhT = np.ascontiguousarray((0.5 * W).T)
    bcols = np.stack([b, -2.0 * b], axis=1).astype(np.float32)
    b_bcast = np.broadcast_to(b.reshape(1, D), (P, D)).astype(np.float32)
    b_bcast = np.ascontiguousarray(b_bcast)
    ones_mat = np.ones((P, P), np.float32)

    in_maps = []
    for i in range(B):
        in_maps.append({
            "xT": np.ascontiguousarray(x[i].T),
            "WT": WT,
            "Wm2T": Wm2T,
            "WhT": WhT,
            "bcols": bcols,
            "b_bcast": b_bcast,
            "ones_mat": ones_mat,
        })

    trace = bool(int(os.environ.get("KBENCH_TRACE", "0")))
    kres = run_bass_kernel_spmd(nc, in_maps, list(range(B)), trace=trace)
    _NC_CACHE["last_result"] = kres
    res = kres.results

    q = np.stack([np.ascontiguousarray(r["qT"].T) for r in res])
    k = np.stack([np.ascontiguousarray(r["kT"].T) for r in res])
    return q, k, k


# revision 31
# speedup vs baseline: 4.0394x; 4.0394x over previous
"""Trainium2 Bass kernel for EuclideanSimilarity (retrieval_knn).

Reference computation per batch b (B=8, L=4096, D=128):
    projected = x @ W.T + b                      [L, D]
    q = avgpool2(x) @ W.T + b                    [L/2, D]   (== avgpool2(projected))
    power = ||q_i||^2 + ||k_j||^2 - 2 q_i.k_j    [L/2, L]
    sim = exp(-sqrt(max(power, 0)))
    k = sim @ projected                          [L/2, D]
    returns (q, k, v=k)

Sharding: data-parallel over batch, one batch element per NeuronCore (8 cores).
All device tensors keep the feature dim D=128 on SBUF partitions where the
matmuls contract over it; host pre-transposes x and post-transposes q/k
(host-side layout prep is free wrt HW exec time).

Per-core pipeline (all fp32):
  projT_m2[e,l] = (-2W)^T x + (-2b)      8 matmuls, stationary -2W^T
  projnat[l,e]  = x_tile^T W^T + b       32 matmuls, xT tiles stationary
  ksq[j]        = sum_e projnat^2        DVE square+reduce
  qT[e,i]       = (W/2)^T (x_even+x_odd) + b
  qsq_bcast     = ones^T @ qT^2          reduce+partition-broadcast in one matmul
  per 1024-query chunk:
    psum  = -2 q.k                       (GEMM2, projT_m2 tiles stationary)
    power = psum + ksq[j] + qsq[i]       one fused DVE op (affine_then_add)
    sim   = Exp(-Sqrt(power))            two ACT passes over 32K-column strip
    kT   += projnat_jt @ sim_strip       (GEMM3, accumulated over 32 j-tiles)
"""

import os
import sys

for _p in ("/opt/trn_rl_repo", "/root/.axon_site/_ro/trn_rl_repo"):
    if os.path.isdir(_p) and _p not in sys.path:
        sys.path.insert(0, _p)

import numpy as np

import concourse.bass as bass
import concourse.mybir as mybir
from concourse import bacc
from concourse.bass_utils import run_bass_kernel_spmd
from concourse.tile import TileContext

B, L, D = 8, 4096, 128
LQ = L // 2          # 2048 pooled queries
P = 128              # partitions
NI = 512             # i-chunk (queries per chunk)
NCHUNK = LQ // NI    # 4
NJT = L // P         # 32 j-tiles
F32 = mybir.dt.float32
F32R = mybir.dt.float32r

# KMODE:
#   f32   - everything fp32 (~4e-7 rel err, fp32 matmul is 4 cyc/row)
#   f32r2 - GEMM2 (qk) operands float32r (~4e-5 rel err, GEMM2 4x faster)
#   f32r  - GEMM2+GEMM3 float32r (~1.2e-4 rel err, both GEMMs 4x faster)
KMODE = os.environ.get("KMODE", "f32r")

AF = mybir.ActivationFunctionType
ALU = mybir.AluOpType


def build_nc(repeat=1, mode=None):
    mode = KMODE if mode is None else mode
    g2r = mode in ("f32r", "f32r2")   # GEMM2 operands f32r
    g3r = mode == "f32r"              # GEMM3 operands f32r
    G2DT = F32R if g2r else F32
    G3DT = F32R if g3r else F32
    nc = bacc.Bacc("TRN2", target_bir_lowering=False)

    xT = nc.declare_dram_parameter("xT", [P, L], F32, isOutput=False)
    WT = nc.declare_dram_parameter("WT", [P, D], F32, isOutput=False)       # W.T
    Wm2T = nc.declare_dram_parameter("Wm2T", [P, D], F32, isOutput=False)   # (-2W).T
    WhT = nc.declare_dram_parameter("WhT", [P, D], F32, isOutput=False)     # (0.5W).T
    bcols = nc.declare_dram_parameter("bcols", [P, 2], F32, isOutput=False)  # [b, -2b]
    b_bcast_in = nc.declare_dram_parameter("b_bcast", [P, D], F32, isOutput=False)
    ones_in = nc.declare_dram_parameter("ones_mat", [P, P], F32, isOutput=False)

    qT_out = nc.declare_dram_parameter("qT", [P, LQ], F32, isOutput=True)
    kT_out = nc.declare_dram_parameter("kT", [P, LQ], F32, isOutput=True)

    with TileContext(nc) as tc:
      for _rep in range(repeat):
        with (
            tc.tile_pool(name="consts", bufs=1) as consts,
            tc.tile_pool(name="big", bufs=1) as big,
            tc.tile_pool(name="work", bufs=2) as work,
            tc.tile_pool(name="ps1", bufs=2, space="PSUM") as ps1,
        ):
            # ---- constants ----
            WT_sb = consts.tile([P, D], F32)
            Wm2T_sb = consts.tile([P, D], F32)
            WhT_sb = consts.tile([P, D], F32)
            bcols_sb = consts.tile([P, 2], F32)
            b_bcast = consts.tile([P, D], F32)
            ones_sb = consts.tile([P, P], F32)
            nc.sync.dma_start(out=WT_sb[:], in_=WT[:])
            nc.sync.dma_start(out=Wm2T_sb[:], in_=Wm2T[:])
            nc.sync.dma_start(out=WhT_sb[:], in_=WhT[:])
            nc.sync.dma_start(out=bcols_sb[:], in_=bcols[:])
            nc.sync.dma_start(out=b_bcast[:], in_=b_bcast_in[:])
            nc.sync.dma_start(out=ones_sb[:], in_=ones_in[:])
            b_col = bcols_sb[:, 0:1]
            bm2_col = bcols_sb[:, 1:2]

            projTm2 = big.tile([P, L], G2DT)   # GEMM2 stationary operand
            projnat = big.tile([P, L], G3DT)   # GEMM3 stationary operand
            qT_sb = big.tile([P, LQ], F32)
            if g2r:
                qT_mm = big.tile([P, LQ], G2DT, tag="qT_mm", name="qT_mm")
            else:
                qT_mm = qT_sb
            qsq_bcast = big.tile([P, LQ], F32)
            ksq = consts.tile([P, NJT], F32)

            # ---- phase 1 (xT-dependent); pool closed before strip opens ----
            with tc.tile_pool(name="phase1", bufs=1) as ph1:
                xT_sb = ph1.tile([P, L], F32)
                nc.sync.dma_start(out=xT_sb[:], in_=xT[:])
                xT_pairs = xT_sb.rearrange("p (i two) -> p i two", two=2)
                xpool = ph1.tile([P, LQ], F32)
                nc.gpsimd.tensor_add(xpool[:], xT_pairs[:, :, 0], xT_pairs[:, :, 1])

                # projT_m2[e, l] = -2 * (W x + b)^T
                for c in range(L // 512):
                    ps = ps1.tile([P, 512], F32, tag="ps1")
                    nc.tensor.matmul(
                        ps, Wm2T_sb[:], xT_sb[:, c * 512:(c + 1) * 512],
                        start=True, stop=True,
                    )
                    nc.vector.tensor_scalar_add(
                        projTm2[:, c * 512:(c + 1) * 512], ps, bm2_col)

                # proj_nat tiles [l(128), e] (rounded to MMDT) and fp32 ksq
                for t in range(NJT):
                    ps = ps1.tile([P, D], F32, tag="ps1")
                    nc.tensor.matmul(
                        ps, xT_sb[:, t * P:(t + 1) * P], WT_sb[:],
                        start=True, stop=True,
                    )
                    if g3r:
                        seg32 = work.tile([P, D], F32, tag="sqs")
                        nc.vector.tensor_add(seg32[:], ps, b_bcast[:])
                        nc.vector.tensor_copy(
                            projnat[:, t * P:(t + 1) * P], seg32[:])
                    else:
                        seg32 = projnat[:, t * P:(t + 1) * P]
                        nc.vector.tensor_add(seg32, ps, b_bcast[:])
                    sq = work.tile([P, D], F32, tag="sqs")
                    nc.gpsimd.tensor_mul(sq[:], seg32[:], seg32[:])
                    nc.vector.tensor_reduce(
                        ksq[:, t:t + 1], sq[:], mybir.AxisListType.X, ALU.add)

                # qT[e, i] = 0.5*W @ xpool + b
                for c in range(LQ // 512):
                    ps = ps1.tile([P, 512], F32, tag="ps1")
                    nc.tensor.matmul(
                        ps, WhT_sb[:], xpool[:, c * 512:(c + 1) * 512],
                        start=True, stop=True,
                    )
                    nc.scalar.add(qT_sb[:, c * 512:(c + 1) * 512], ps, b_col)
                nc.sync.dma_start(out=qT_out[:], in_=qT_sb[:])
                if g2r:
                    nc.gpsimd.tensor_copy(qT_mm[:], qT_sb[:])

                # qsq_bcast[p, i] = ||q_i||^2 broadcast to all partitions:
                # all-ones stationary does reduce + broadcast in one matmul.
                sq_qT = ph1.tile([P, LQ], F32)
                nc.gpsimd.tensor_mul(sq_qT[:], qT_sb[:], qT_sb[:])
                for c in range(LQ // 512):
                    ps = ps1.tile([P, 512], F32, tag="ps1")
                    nc.tensor.matmul(
                        ps, ones_sb[:], sq_qT[:, c * 512:(c + 1) * 512],
                        start=True, stop=True,
                    )
                    nc.scalar.copy(qsq_bcast[:, c * 512:(c + 1) * 512], ps)

            # ---- main loop over query chunks ----
            with (
                tc.tile_pool(name="stripp", bufs=1 if g3r else 2) as stripp,
                tc.tile_pool(name="psqk", bufs=4, space="PSUM") as psqk,
                tc.tile_pool(name="psk", bufs=2, space="PSUM") as psk,
            ):
                for c in range(NCHUNK):
                    strip = stripp.tile([P, NJT * NI], F32, tag="strip")
                    if g3r:
                        sim = stripp.tile(
                            [P, NJT * NI], F32R, tag="sim", name="sim")
                    else:
                        sim = strip
                    qs = qsq_bcast[:, c * NI:(c + 1) * NI]
                    qchunk = qT_mm[:, c * NI:(c + 1) * NI]
                    for jt in range(NJT):
                        ps2 = psqk.tile([P, NI], F32, tag="qk")
                        nc.tensor.matmul(
                            ps2, projTm2[:, jt * P:(jt + 1) * P], qchunk,
                            start=True, stop=True,
                        )
                        # power = (-2qk) + ksq[j] + qsq[i], one fused DVE op
                        nc.vector.affine_then_add(
                            strip[:, jt * NI:(jt + 1) * NI], ps2, qs,
                            scale=1.0, bias=ksq[:, jt:jt + 1],
                        )
                    nc.scalar.activation(strip[:], strip[:], AF.Sqrt)
                    nc.scalar.activation(sim[:], strip[:], AF.Exp, scale=-1.0)
                    ps3 = psk.tile([P, NI], F32, tag="kacc")
                    for jt in range(NJT):
                        nc.tensor.matmul(
                            ps3, projnat[:, jt * P:(jt + 1) * P],
                            sim[:, jt * NI:(jt + 1) * NI],
                            start=(jt == 0), stop=(jt == NJT - 1),
                        )
                    kT_tile = work.tile([P, NI], F32, tag="kout")
                    nc.vector.tensor_copy(kT_tile[:], ps3)
                    nc.sync.dma_start(
                        out=kT_out[:, c * NI:(c + 1) * NI], in_=kT_tile[:])

    nc.compile()
    return nc


_NC_CACHE = {}


def _get_nc():
    key = ("nc", KMODE)
    if key not in _NC_CACHE:
        _NC_CACHE[key] = build_nc()
    return _NC_CACHE[key]


def kernel(x, W, b):
    x = np.asarray(x, dtype=np.float32)
    W = np.asarray(W, dtype=np.float32)
    b = np.asarray(b, dtype=np.float32)

    nc = _get_nc()

    WT = np.ascontiguousarray(W.T)
    Wm2T = np.ascontiguousarray((-2.0 * W).T)
    WhT = np.ascontiguousarray((0.5 * W).T)
    bcols = np.stack([b, -2.0 * b], axis=1).astype(np.float32)
    b_bcast = np.broadcast_to(b.reshape(1, D), (P, D)).astype(np.float32)
    b_bcast = np.ascontiguousarray(b_bcast)
    ones_mat = np.ones((P, P), np.float32)

    in_maps = []
    for i in range(B):
        in_maps.append({
            "xT": np.ascontiguousarray(x[i].T),
            "WT": WT,
            "Wm2T": Wm2T,
            "WhT": WhT,
            "bcols": bcols,
            "b_bcast": b_bcast,
            "ones_mat": ones_mat,
        })

    trace = bool(int(os.environ.get("KBENCH_TRACE", "0")))
    kres = run_bass_kernel_spmd(nc, in_maps, list(range(B)), trace=trace)
    _NC_CACHE["last_result"] = kres
    res = kres.results

    q = np.stack([np.ascontiguousarray(r["qT"].T) for r in res])
    k = np.stack([np.ascontiguousarray(r["kT"].T) for r in res])
    return q, k, k
